# revision 36
# baseline (speedup 1.0000x reference)
"""GroupedQueryAttention forward on 8 Trainium2 NeuronCores (Bass/Tile), v4.

Sharding (per spec hint): data-parallel over batch (B=2) x tensor-parallel
over KV-head groups (4 groups of 2 KV heads + their 8 query heads each).
Core c -> (batch b = c // 4, group g = c % 4).

v4 changes vs v2 (377us):
  - input DMA spread over 3 hw queues (sync: xT; scalar: wk+wq halves;
    gpsimd: wv+msk+wo) with first-needed tiles at each ring head, and
    the j=0 projection emission staggered so the k/v chains (whose
    weights land first) lead the q chains by two chunk-steps.
  - PE HAM pre-warm: dummy matmuls into a spare PSUM bank while the
    first input tiles are in flight.
  - causal mask applied with one strided 3D-AP tensor_mul per head pair
    against a replicated mask tile (halves DVE MULTIPLY count).
  - all PSUM evacuations (aot/rbr) moved off the scalar engine to DVE;
    scalar runs (almost) only the EXPs.
  - o_proj output staged in a [128,2048] bf16 tile per row-block and
    DMA'd as fully contiguous 4KB DRAM rows, row-split across the
    gpsimd and sync queues; tail rotates po over 6 free PSUM slots so
    the last chunk's o_proj drains without stalls.

All device compute bf16 with fp32 PSUM accumulation. Host pre-casts/
pre-transposes x and pre-slices and pair-reorders the weight shards.
"""

from collections import deque

import numpy as np

import concourse.bass as bass  # noqa: F401  (import keeps engine registry warm)
import concourse.mybir as mybir
import concourse.tile as tile
from concourse import bacc, bass_utils
from concourse.ap import AP

# Problem shape (hardcoded per contract).
B, N, D = 2, 2048, 2048
NUM_HEADS = 32
NUM_KV_HEADS = 8
HD = 64
G = NUM_HEADS // NUM_KV_HEADS  # 4
N_CORES = 8
NT = D // 128                  # 16 contraction tiles
NCHUNK = 4                     # token chunks of 512
CH = 512

_CACHE = {}


def _build():
    nc = bacc.Bacc("TRN2", target_bir_lowering=False, debug=False,
                   num_devices=N_CORES)
    f32, bf16 = mybir.dt.float32, mybir.dt.bfloat16
    Exp = mybir.ActivationFunctionType.Exp

    # all inputs pre-arranged host-side into the SBUF tile layout
    # (partition-major) so every load is contiguous per partition
    xT = nc.dram_tensor("xT", [128, NT * N], bf16, kind="ExternalInput")
    wqA = nc.dram_tensor("wqA", [128, NT * 256], bf16, kind="ExternalInput")
    wqB = nc.dram_tensor("wqB", [128, NT * 256], bf16, kind="ExternalInput")
    wk = nc.dram_tensor("wk", [128, NT * 128], bf16, kind="ExternalInput")
    wv = nc.dram_tensor("wv", [128, NT * 128], bf16, kind="ExternalInput")
    wo = nc.dram_tensor("wo", [128, 4 * D], bf16, kind="ExternalInput")
    msk = nc.dram_tensor("msk", [128, 512], bf16, kind="ExternalInput")
    part = nc.dram_tensor("part", [N, D], bf16, kind="ExternalOutput")

    with tile.TileContext(nc) as tc:
        with (
            tc.tile_pool(name="const", bufs=1) as cpool,
            tc.tile_pool(name="proj", bufs=1) as kpool,
            tc.tile_pool(name="work", bufs=2) as wpool,
            tc.tile_pool(name="ps_s", bufs=1, space="PSUM") as ps_s,
            tc.tile_pool(name="ps_pa", bufs=1, space="PSUM") as ps_pa,
            tc.tile_pool(name="ps_d", bufs=1, space="PSUM") as ps_d,
            tc.tile_pool(name="ps_proj", bufs=1, space="PSUM") as ps_proj,
        ):
            # ---- inputs / constants -------------------------------------
            # ring plan: sync carries x^T (first chunk at the ring head);
            # scalar carries wk + the two wq halves; gpsimd carries wv,
            # msk and the late-needed wo.
            xtc = []
            for t in range(NT):
                xq = cpool.tile([128, N], bf16, tag=f"xtc{t}")
                nc.sync.dma_start(xq[:], xT.ap()[:, t * N:(t + 1) * N])
                xtc.append(xq)
            wk_t = cpool.tile([128, NT * 128], bf16, tag="wk")
            nc.scalar.dma_start(wk_t[:], wk.ap()[:])
            wqA_t = cpool.tile([128, NT * 256], bf16, tag="wqA")
            nc.scalar.dma_start(wqA_t[:], wqA.ap()[:])
            wqB_t = cpool.tile([128, NT * 256], bf16, tag="wqB")
            nc.scalar.dma_start(wqB_t[:], wqB.ap()[:])
            wv_t = cpool.tile([128, NT * 128], bf16, tag="wv")
            nc.gpsimd.dma_start(wv_t[:], wv.ap()[:])
            msk_t = cpool.tile([128, 512], bf16, tag="msk")
            nc.gpsimd.dma_start(msk_t[:], msk.ap()[:])
            # wo rides the END of the sync ring: it is only needed ~100us
            # in, and keeping it off the early ring mix gives x^T the
            # full early bandwidth.
            wo_t = cpool.tile([128, 4 * D], bf16, tag="wo")
            nc.sync.dma_start(wo_t[:], wo.ap()[:])

            def xt_rhs(t, j):
                return xtc[t][:, j * CH:(j + 1) * CH]

            def wq_col(t, a):
                wt = wqA_t if a < 2 else wqB_t
                return wt[:, t * 256 + (a % 2) * 128:
                          t * 256 + (a % 2 + 1) * 128]
            ones1 = cpool.tile([128, 1], bf16, tag="ones1")
            nc.vector.memset(ones1[:], 1.0)
            onesb = cpool.tile([128, 64], f32, tag="onesb")
            nc.vector.memset(onesb[:], 1.0)
            warm = cpool.tile([128, 256], bf16, tag="warm")
            nc.vector.memset(warm[:], 0.0)
            # pre-warm the exp activation table while the DMAs run
            scr = cpool.tile([1, 8], f32, tag="scr")
            nc.vector.memset(scr[:], 0.0)
            nc.scalar.activation(scr[0:1, :], scr[0:1, :], Exp)

            # HAM pre-warm: dummy matmuls into the (otherwise idle) second
            # bank of the j=0 s0 tile keep the PE activity window busy so
            # the real chains start at 2.4GHz.
            s0_j0 = ps_s.tile([128, 1024], f32, tag="s0", name="s0j0")
            s1_j0 = ps_s.tile([128, 1024], f32, tag="s1", name="s1j0")
            for _ in range(10):
                nc.tensor.matmul(s0_j0[0:1, CH:CH + 256], ones1[:],
                                 warm[:], start=True, stop=True)

            # persistent projection outputs
            kt = [kpool.tile([128, CH], bf16, tag=f"kt{j}", name=f"kt{j}")
                  for j in range(NCHUNK)]
            v3 = [kpool.tile([128, 128], bf16, tag=f"v3_{m}", name=f"v3_{m}")
                  for m in range(N // 128)]
            qt = [[kpool.tile([128, CH], bf16, tag=f"qt{a}_{j}",
                              name=f"qt{a}_{j}")
                   for j in range(NCHUNK)] for a in range(4)]

            # ---- filler machinery ---------------------------------------
            fillers = deque()  # (emit_fn, est_pe_ns)

            def drain(budget_ns):
                spent = 0
                while fillers and spent < budget_ns:
                    fn, est = fillers.popleft()
                    fn()
                    spent += est

            def drain_all():
                while fillers:
                    fn, _ = fillers.popleft()
                    fn()

            # ---- projection thunks --------------------------------------
            def proj_chain(dst_evac, lhsT_col, j, alloc_ps):
                """Returns thunk fns for one 16-deep contraction chain.

                lhsT_col(t) -> AP for the stationary tile;
                dst_evac(ps) emits the evacuation."""
                ps_box = {}

                def pair(q):
                    def emit():
                        if q == 0:
                            ps_box["ps"] = alloc_ps()
                        ps = ps_box["ps"]
                        for t in range(2 * q, 2 * q + 2):
                            nc.tensor.matmul(
                                ps[:], lhsT_col(t), xt_rhs(t, j),
                                start=(t == 0), stop=(t == NT - 1))
                        if q == 7:
                            dst_evac(ps)
                    return emit
                return [(pair(q), 440) for q in range(8)]

            def proj_thunks(j):
                # j=0 runs before any attention: rotate the 6 chains over
                # the (then free) attention banks and emit chunk-major —
                # with the k/v chains leading the q chains by two steps
                # (wq lands on the scalar ring after wk).
                if j == 0:
                    rots = [
                        lambda: ps_proj.tile([128, CH], f32, tag="proj",
                                             name="pps"),
                        lambda: ps_pa.tile([128, CH], f32, tag="paA",
                                           name="pps"),
                        lambda: ps_pa.tile([128, CH], f32, tag="paB",
                                          name="pps"),
                        lambda: ps_d.tile([128, CH], f32, tag="dn",
                                          name="pps"),
                        lambda: s0_j0[:, 0:CH],
                        lambda: s1_j0[:, 0:CH],
                    ]
                else:
                    rots = [lambda: ps_proj.tile([128, CH], f32, tag="proj",
                                                 name="pps")]
                ch = [0]

                def nxt():
                    a = rots[ch[0] % len(rots)]
                    ch[0] += 1
                    return a
                chains = []

                def kev(ps):
                    nc.vector.tensor_copy(kt[j][:], ps[:])
                chains.append(proj_chain(
                    kev, lambda t: wk_t[:, t * 128:(t + 1) * 128], j, nxt()))

                def vev(ps):
                    vt_s = wpool.tile([128, CH], bf16, tag="vt", name="vt_s")
                    nc.vector.tensor_copy(vt_s[:], ps[:])
                    for s in range(4):
                        nc.sync.dma_start_transpose(
                            v3[4 * j + s][:], vt_s[:, s * 128:(s + 1) * 128])
                chains.append(proj_chain(
                    vev, lambda t: wv_t[:, t * 128:(t + 1) * 128], j, nxt()))
                for a in range(4):
                    def qev(ps, a=a):
                        nc.vector.tensor_copy(qt[a][j][:], ps[:])
                    chains.append(proj_chain(
                        qev, lambda t, a=a: wq_col(t, a), j, nxt()))
                if j == 0:
                    # staggered chunk-major: k/v advance at DMA pace, the
                    # q chains trail by 2 chunk-steps.
                    seq = []
                    for q in range(8):
                        seq.append(chains[0][q])
                        seq.append(chains[1][q])
                        if q >= 2:
                            seq.extend(chains[c][q - 2] for c in range(2, 6))
                    for q in (6, 7):
                        seq.extend(chains[c][q] for c in range(2, 6))
                    return seq
                kv = [th for chain in chains[0:2] for th in chain]
                qs = [th for chain in chains[2:6] for th in chain]
                return kv, qs

            # ---- o_proj thunks ------------------------------------------
            # last chunk: the a=0,1 half-contraction runs as fillers
            # during g=1 of attn(3) and streams straight to DRAM; the
            # tail computes a=2,3 and ACCUMULATES into the same DRAM rows
            # via gpsimd DMA (same ring -> ordered), halving the
            # post-attention PE run.
            def oproj_half_thunks(ci, an_tiles):
                n0 = ci * CH
                th = []
                st_box = {}
                for nt_ in range(4):
                    for dc in range(4):
                        def emit(nt_=nt_, dc=dc):
                            po = ps_proj.tile([128, CH], f32, tag="proj",
                                              name="poh")
                            for a in range(2):
                                nc.tensor.matmul(
                                    po[:],
                                    an_tiles[a][:, nt_ * 128:(nt_ + 1) * 128],
                                    wo_t[:, a * D + dc * CH:
                                         a * D + (dc + 1) * CH],
                                    start=(a == 0), stop=(a == 1))
                            if dc == 0:
                                st_box[nt_] = wpool.tile(
                                    [128, 2048], bf16, tag="st", bufs=2,
                                    name="sth")
                            st = st_box[nt_]
                            nc.vector.tensor_copy(
                                st[:, dc * CH:(dc + 1) * CH], po[:])
                            if dc == 3:
                                r0 = n0 + nt_ * 128
                                nc.gpsimd.dma_start(
                                    part.ap()[r0:r0 + 128, :], st[:])
                        th.append((emit, 470))
                return th

            def oproj_thunks(ci, an_tiles, tail=False):
                n0 = ci * CH
                th = []
                # in the tail (post-attention) phase all attention banks
                # are free: rotate po over 6 slots so the PE never waits
                # on a single bank's evacuation.
                tail_rots = [
                    lambda: ps_proj.tile([128, CH], f32, tag="proj",
                                         name="po"),
                    lambda: ps_pa.tile([128, CH], f32, tag="paA", name="po"),
                    lambda: ps_pa.tile([128, CH], f32, tag="paB", name="po"),
                    lambda: ps_d.tile([128, CH], f32, tag="dn", name="po"),
                    lambda: ps_s.tile([128, 1024], f32, tag="s0",
                                      name="po")[:, 0:CH],
                    lambda: ps_s.tile([128, 1024], f32, tag="s1",
                                      name="po")[:, 0:CH],
                ]
                st_box = {}
                for nt_ in range(4):
                    for dc in range(4):
                        def emit(nt_=nt_, dc=dc):
                            if tail:
                                po = tail_rots[(nt_ * 4 + dc) % 6]()
                            else:
                                po = ps_proj.tile([128, CH], f32, tag="proj",
                                                  name="po")
                            a_lo = 2 if tail else 0
                            for a in range(a_lo, 4):
                                nc.tensor.matmul(
                                    po[:],
                                    an_tiles[a][:, nt_ * 128:(nt_ + 1) * 128],
                                    wo_t[:, a * D + dc * CH:
                                         a * D + (dc + 1) * CH],
                                    start=(a == a_lo), stop=(a == 3))
                            if dc == 0:
                                st_box[nt_] = wpool.tile(
                                    [128, 2048], bf16, tag="st", bufs=2,
                                    name="st")
                            st = st_box[nt_]
                            dst = st[:, dc * CH:(dc + 1) * CH]
                            # tail: split the copy across scalar+vector so
                            # the final accumulate-DMAs fire early;
                            # mid-kernel: keep scalar free for the EXPs.
                            if tail:
                                nc.scalar.copy(dst[0:64, :], po[0:64, :])
                                nc.vector.tensor_copy(dst[64:128, :],
                                                      po[64:128, :])
                            else:
                                nc.vector.tensor_copy(dst, po[:])
                            if dc == 3:
                                r0 = n0 + nt_ * 128
                                if tail:
                                    # adds the staged a=0,1 half already
                                    # in DRAM (same gpsimd ring -> ordered)
                                    nc.gpsimd.dma_start(
                                        part.ap()[r0:r0 + 128, :], st[:],
                                        accum_op=mybir.AluOpType.add)
                                else:
                                    nc.gpsimd.dma_start(
                                        part.ap()[r0:r0 + 64, :],
                                        st[0:64, :])
                                    nc.sync.dma_start(
                                        part.ap()[r0 + 64:r0 + 128, :],
                                        st[64:128, :])
                        th.append((emit, 880))
                return th

            # ---- attention ----------------------------------------------
            def attn_chunk(ci, mid_hook=None):
                M = 4 * ci + 4
                an_tiles = []
                for g in range(2):
                    # spread the queued filler work evenly over the
                    # remaining drain points (one per (g, mt) block) so
                    # fillers run in long PE-dense runs and never dry up
                    # mid-chunk.
                    tot_est = sum(e for _, e in fillers)
                    per_block = tot_est / ((2 - g) * M) if tot_est else 0
                    a0, a1 = 2 * g, 2 * g + 1
                    paA = ps_pa.tile([128, CH], f32, tag="paA", name="paA")
                    paB = ps_pa.tile([128, CH], f32, tag="paB", name="paB")
                    dn = ps_d.tile([128, CH], f32, tag="dn", name="dn")
                    pts = {}

                    def scores_exp(mt):
                        # score layout per head pair a: kv0 block at cols
                        # [0:F], kv1 block at cols [512:512+F] (bank 2);
                        # one FD=512+F exp covers both (cols [F:512] are
                        # junk for diagonal tiles and never read).
                        jmt, cmt = mt // 4, mt % 4
                        flo = max(0, (mt - 4 * ci) * 128)
                        F = CH - flo
                        # kv1 block always in bank 1: the two row-tiled
                        # score matmuls run concurrently and must not
                        # target the same PSUM bank (cols [F:CH] junk)
                        off = CH
                        sss, pt_pair = [], []
                        for i, a in enumerate((a0, a1)):
                            ss = ps_s.tile([128, 1024], f32, tag=f"s{i}",
                                           name="ss")
                            nc.tensor.matmul(
                                ss[:, 0:F],
                                kt[jmt][0:64, cmt * 128:(cmt + 1) * 128],
                                qt[a][ci][0:64, flo:CH],
                                start=True, stop=True, tile_position=(0, 0))
                            nc.tensor.matmul(
                                ss[:, off:off + F],
                                kt[jmt][64:128, cmt * 128:(cmt + 1) * 128],
                                qt[a][ci][64:128, flo:CH],
                                start=True, stop=True, tile_position=(64, 0))
                            sss.append(ss)
                        for i in range(2):
                            pt_ = wpool.tile([128, 1024], bf16, tag=f"pt{i}",
                                             name="pt")
                            if F == CH:
                                nc.scalar.activation(pt_[:, 0:off + F],
                                                     sss[i][:, 0:off + F],
                                                     Exp, scale=0.125)
                            else:
                                # diagonal tile: strided AP skips the junk
                                # gap [F:512] so the exp covers only the
                                # two live blocks (scalar chain is the
                                # block-rate limiter)
                                sa, pa_ = sss[i][:], pt_[:]
                                src = AP(sa.tensor, sa.offset,
                                         [[sa.ap[0][0], 128],
                                          [CH, 2], [1, F]])
                                dst = AP(pa_.tensor, pa_.offset,
                                         [[pa_.ap[0][0], 128],
                                          [CH, 2], [1, F]])
                                nc.scalar.activation(dst, src,
                                                     Exp, scale=0.125)
                            if mt >= 4 * ci:  # diagonal block: causal mask
                                pta = pt_[:]
                                pv = AP(pta.tensor, pta.offset,
                                        [[1024, 128], [512, 2], [1, 128]])
                                ma = msk_t[:]
                                mv = AP(ma.tensor, ma.offset,
                                        [[512, 128], [128, 2], [1, 128]])
                                nc.vector.tensor_mul(pv, pv, mv)
                            pt_pair.append(pt_)
                        pts[mt] = (pt_pair, flo, F, off)

                    def av_denom(mt):
                        pt_pair, flo, F, off = pts.pop(mt)
                        first, last = (mt == 0), (mt == M - 1)
                        for i in range(2):
                            pa = paA if i == 0 else paB
                            pt_ = pt_pair[i]
                            nc.tensor.matmul(
                                pa[0:64, flo:CH], v3[mt][:, 0:64],
                                pt_[:, 0:F],
                                start=first, stop=last, tile_position=(0, 0))
                            nc.tensor.matmul(
                                pa[64:128, flo:CH], v3[mt][:, 64:128],
                                pt_[:, off:off + F],
                                start=first, stop=last, tile_position=(0, 64))
                        for pos, src in ((0, pt_pair[0][:, 0:F]),
                                         (32, pt_pair[0][:, off:off + F]),
                                         (64, pt_pair[1][:, 0:F]),
                                         (96, pt_pair[1][:, off:off + F])):
                            nc.tensor.matmul(
                                dn[pos:pos + 1, flo:CH], ones1[:], src,
                                start=first, stop=last,
                                tile_position=(0, pos))

                    for mt in range(M):
                        scores_exp(mt)
                        if mt > 0:
                            av_denom(mt - 1)
                        drain(max(900, per_block))
                    av_denom(M - 1)

                    # Evacuate pa raw (frees the accumulator banks for the
                    # next group immediately); normalize off-critical-path.
                    aots = []
                    for i in range(2):
                        aot = wpool.tile([128, CH], bf16, tag=f"aot{i}",
                                         name="aot")
                        nc.vector.tensor_copy(aot[:], (paA if i == 0
                                                       else paB)[:])
                        aots.append(aot)
                    d4r = wpool.tile([128, CH], f32, tag="d4r", name="d4r")
                    nc.vector.reciprocal_approx_fast(d4r[0:97, :], dn[0:97, :])
                    # broadcast 1/denom rows to 128 partitions via K=1 PE
                    # matmuls into the (now free) score banks: all four in
                    # one window (pairwise-disjoint PE quadrants)
                    rbs = [ps_s.tile([128, 1024], f32, tag=f"s{i}",
                                     name="rb") for i in range(2)]
                    for i in range(2):
                        r0, r1 = 64 * i, 64 * i + 32
                        nc.tensor.matmul(rbs[i][0:64, 0:CH],
                                         onesb[r0:r0 + 1, :],
                                         d4r[r0:r0 + 1, :], start=True,
                                         stop=True, tile_position=(r0, 0))
                        nc.tensor.matmul(rbs[i][64:128, CH:2 * CH],
                                         onesb[r1:r1 + 1, :],
                                         d4r[r1:r1 + 1, :], start=True,
                                         stop=True, tile_position=(r1, 64))
                    for i, a in enumerate((a0, a1)):
                        rbr = wpool.tile([128, CH], f32, tag=f"rbr{i}",
                                         name="rbr")
                        nc.vector.tensor_copy(rbr[0:64, :],
                                              rbs[i][0:64, 0:CH])
                        nc.vector.tensor_copy(rbr[64:128, :],
                                              rbs[i][64:128, CH:2 * CH])
                        an = wpool.tile([128, CH], bf16, tag=f"an{a}",
                                        name=f"an{a}")
                        nc.vector.tensor_mul(an[:], aots[i][:], rbr[:])
                        an_tiles.append(an)
                    if g == 0 and mid_hook is not None:
                        fillers.extend(mid_hook(an_tiles))
                return an_tiles

            # ---- main schedule ------------------------------------------
            for fn, _ in proj_thunks(0):
                fn()
            def mix3(pth, oth):
                # interleave proj and oproj fillers ~3:1 so each oproj
                # emit's PSUM evacuation hides under proj matmuls instead
                # of stalling the single o_proj bank.
                mix, pi, oi = [], 0, 0
                while pi < len(pth) or oi < len(oth):
                    for _ in range(3):
                        if pi < len(pth):
                            mix.append(pth[pi])
                            pi += 1
                    if oi < len(oth):
                        mix.append(oth[oi])
                        oi += 1
                return mix

            # filler placement: q-chains of chunk j must finish before
            # attn(j) starts, but the k/v outputs of chunk 3 are only
            # consumed from mt=12 onward — defer them into ci=3, which is
            # otherwise filler-poor (exp-gated blocks would starve the PE).
            kv1, qs1 = proj_thunks(1)
            kv2, qs2 = proj_thunks(2)
            kv3, qs3 = proj_thunks(3)
            an_by_ci = {}
            for ci in range(NCHUNK):
                if ci == 0:
                    fillers.extend(mix3(qs1 + kv1, []))
                elif ci == 1:
                    fillers.extend(mix3(qs2 + kv2,
                                        oproj_thunks(0, an_by_ci[0])))
                elif ci == 2:
                    fillers.extend(mix3(qs3,
                                        oproj_thunks(1, an_by_ci[1])))
                else:
                    fillers.extend(mix3(kv3,
                                        oproj_thunks(2, an_by_ci[2])))
                mid = (lambda an2: oproj_half_thunks(NCHUNK - 1, an2)) \
                    if ci == NCHUNK - 1 else None
                an_by_ci[ci] = attn_chunk(ci, mid_hook=mid)
                drain_all()
            for fn, _ in oproj_thunks(NCHUNK - 1, an_by_ci[NCHUNK - 1],
                                      tail=True):
                fn()
    nc.compile()
    return nc


def _prep_in_maps(x, Wq, Wk, Wv, Wo):
    import jax.numpy as jnp

    def to_bf16(a):
        return np.asarray(jnp.asarray(np.asarray(a), dtype=jnp.bfloat16))

    i = np.arange(128)[:, None]
    j = np.arange(128)[None, :]
    msk = np.tile((i <= j).astype(np.float32), (1, 4))

    def devlay(a):
        # [K*128, O] -> [128, K*O] partition-major device layout
        k = a.shape[0] // 128
        return a.reshape(k, 128, a.shape[1]).transpose(1, 0, 2).reshape(128, -1)

    in_maps = []
    for c in range(N_CORES):
        b, g = c // 4, c % 4
        qh = [8 * g + a for a in range(8)]
        wq_cols = []
        for a in range(4):
            wq_cols.append(np.arange(qh[a] * HD, (qh[a] + 1) * HD))
            wq_cols.append(np.arange(qh[a + 4] * HD, (qh[a + 4] + 1) * HD))
        wq_r = np.asarray(Wq)[:, np.concatenate(wq_cols)]
        wo_r = np.asarray(Wo)[np.concatenate(wq_cols), :]
        wk_s = np.asarray(Wk)[:, 2 * g * HD: (2 * g + 2) * HD]
        wv_s = np.asarray(Wv)[:, 2 * g * HD: (2 * g + 2) * HD]
        in_maps.append({
            "xT": to_bf16(devlay(np.ascontiguousarray(np.asarray(x)[b].T))),
            "wqA": to_bf16(devlay(wq_r[:, 0:256])),
            "wqB": to_bf16(devlay(wq_r[:, 256:512])),
            "wk": to_bf16(devlay(wk_s)),
            "wv": to_bf16(devlay(wv_s)),
            "wo": to_bf16(devlay(wo_r)),
            "msk": to_bf16(msk),
        })
    return in_maps


def kernel(x, Wq, Wk, Wv, Wo, trace=False):
    if "nc" not in _CACHE:
        _CACHE["nc"] = _build()
    nc = _CACHE["nc"]
    in_maps = _prep_in_maps(x, Wq, Wk, Wv, Wo)
    res = bass_utils.run_bass_kernel_spmd(
        nc, in_maps, core_ids=list(range(N_CORES)), trace=trace)
    _CACHE["last_result"] = res
    out = np.zeros((B, N, D), np.float32)
    for c in range(N_CORES):
        out[c // 4] += np.asarray(res.results[c]["part"], dtype=np.float32)
    return out


# revision 39
# speedup vs baseline: 1.0555x; 1.0555x over previous
"""GroupedQueryAttention forward on 8 Trainium2 NeuronCores (Bass/Tile), v4.

Sharding (per spec hint): data-parallel over batch (B=2) x tensor-parallel
over KV-head groups (4 groups of 2 KV heads + their 8 query heads each).
Core c -> (batch b = c // 4, group g = c % 4).

v4 changes vs v2 (377us):
  - input DMA spread over 3 hw queues (sync: xT; scalar: wk+wq halves;
    gpsimd: wv+msk+wo) with first-needed tiles at each ring head, and
    the j=0 projection emission staggered so the k/v chains (whose
    weights land first) lead the q chains by two chunk-steps.
  - PE HAM pre-warm: dummy matmuls into a spare PSUM bank while the
    first input tiles are in flight.
  - causal mask applied with one strided 3D-AP tensor_mul per head pair
    against a replicated mask tile (halves DVE MULTIPLY count).
  - all PSUM evacuations (aot/rbr) moved off the scalar engine to DVE;
    scalar runs (almost) only the EXPs.
  - o_proj output staged in a [128,2048] bf16 tile per row-block and
    DMA'd as fully contiguous 4KB DRAM rows, row-split across the
    gpsimd and sync queues; tail rotates po over 6 free PSUM slots so
    the last chunk's o_proj drains without stalls.

All device compute bf16 with fp32 PSUM accumulation. Host pre-casts/
pre-transposes x and pre-slices and pair-reorders the weight shards.
"""

from collections import deque

import numpy as np

import concourse.bass as bass  # noqa: F401  (import keeps engine registry warm)
import concourse.mybir as mybir
import concourse.tile as tile
from concourse import bacc, bass_utils
from concourse.ap import AP

# Problem shape (hardcoded per contract).
B, N, D = 2, 2048, 2048
NUM_HEADS = 32
NUM_KV_HEADS = 8
HD = 64
G = NUM_HEADS // NUM_KV_HEADS  # 4
N_CORES = 8
NT = D // 128                  # 16 contraction tiles
NCHUNK = 4                     # token chunks of 512
CH = 512

_CACHE = {}


def _build():
    nc = bacc.Bacc("TRN2", target_bir_lowering=False, debug=False,
                   num_devices=N_CORES)
    f32, bf16 = mybir.dt.float32, mybir.dt.bfloat16
    Exp = mybir.ActivationFunctionType.Exp

    # all inputs pre-arranged host-side into the SBUF tile layout
    # (partition-major) so every load is contiguous per partition
    xT = nc.dram_tensor("xT", [128, NT * N], bf16, kind="ExternalInput")
    wqA = nc.dram_tensor("wqA", [128, NT * 256], bf16, kind="ExternalInput")
    wqB = nc.dram_tensor("wqB", [128, NT * 256], bf16, kind="ExternalInput")
    wk = nc.dram_tensor("wk", [128, NT * 128], bf16, kind="ExternalInput")
    wv = nc.dram_tensor("wv", [128, NT * 128], bf16, kind="ExternalInput")
    wo = nc.dram_tensor("wo", [128, 4 * D], bf16, kind="ExternalInput")
    msk = nc.dram_tensor("msk", [128, 512], bf16, kind="ExternalInput")
    part = nc.dram_tensor("part", [N, D], bf16, kind="ExternalOutput")

    with tile.TileContext(nc) as tc:
        with (
            tc.tile_pool(name="const", bufs=1) as cpool,
            tc.tile_pool(name="proj", bufs=1) as kpool,
            tc.tile_pool(name="work", bufs=2) as wpool,
            tc.tile_pool(name="ps_s", bufs=1, space="PSUM") as ps_s,
            tc.tile_pool(name="ps_pa", bufs=1, space="PSUM") as ps_pa,
            tc.tile_pool(name="ps_d", bufs=1, space="PSUM") as ps_d,
            tc.tile_pool(name="ps_proj", bufs=1, space="PSUM") as ps_proj,
        ):
            # ---- inputs / constants -------------------------------------
            # ring plan: sync carries x^T (first chunk at the ring head);
            # scalar carries wk + the two wq halves; gpsimd carries wv,
            # msk and the late-needed wo.
            xtc = []
            for t in range(NT):
                xq = cpool.tile([128, N], bf16, tag=f"xtc{t}")
                nc.sync.dma_start(xq[:], xT.ap()[:, t * N:(t + 1) * N])
                xtc.append(xq)
            wk_t = cpool.tile([128, NT * 128], bf16, tag="wk")
            nc.scalar.dma_start(wk_t[:], wk.ap()[:])
            wqA_t = cpool.tile([128, NT * 256], bf16, tag="wqA")
            nc.scalar.dma_start(wqA_t[:], wqA.ap()[:])
            wqB_t = cpool.tile([128, NT * 256], bf16, tag="wqB")
            nc.scalar.dma_start(wqB_t[:], wqB.ap()[:])
            wv_t = cpool.tile([128, NT * 128], bf16, tag="wv")
            nc.gpsimd.dma_start(wv_t[:], wv.ap()[:])
            msk_t = cpool.tile([128, 512], bf16, tag="msk")
            nc.gpsimd.dma_start(msk_t[:], msk.ap()[:])
            # wo rides the END of the sync ring: it is only needed ~100us
            # in, and keeping it off the early ring mix gives x^T the
            # full early bandwidth.
            wo_t = cpool.tile([128, 4 * D], bf16, tag="wo")
            nc.sync.dma_start(wo_t[:], wo.ap()[:])

            def xt_rhs(t, j):
                return xtc[t][:, j * CH:(j + 1) * CH]

            def wq_col(t, a):
                wt = wqA_t if a < 2 else wqB_t
                return wt[:, t * 256 + (a % 2) * 128:
                          t * 256 + (a % 2 + 1) * 128]
            ones1 = cpool.tile([128, 1], bf16, tag="ones1")
            nc.vector.memset(ones1[:], 1.0)
            onesb = cpool.tile([128, 64], f32, tag="onesb")
            nc.vector.memset(onesb[:], 1.0)
            warm = cpool.tile([128, 256], bf16, tag="warm")
            nc.vector.memset(warm[:], 0.0)
            # pre-warm the exp activation table while the DMAs run
            scr = cpool.tile([1, 8], f32, tag="scr")
            nc.vector.memset(scr[:], 0.0)
            nc.scalar.activation(scr[0:1, :], scr[0:1, :], Exp)

            # HAM pre-warm: dummy matmuls into the (otherwise idle) second
            # bank of the j=0 s0 tile keep the PE activity window busy so
            # the real chains start at 2.4GHz.
            s0_j0 = ps_s.tile([128, 1024], f32, tag="s0", name="s0j0")
            s1_j0 = ps_s.tile([128, 1024], f32, tag="s1", name="s1j0")
            for _ in range(10):
                nc.tensor.matmul(s0_j0[0:1, CH:CH + 256], ones1[:],
                                 warm[:], start=True, stop=True)

            # persistent projection outputs
            kt = [kpool.tile([128, CH], bf16, tag=f"kt{j}", name=f"kt{j}")
                  for j in range(NCHUNK)]
            v3 = [kpool.tile([128, 128], bf16, tag=f"v3_{m}", name=f"v3_{m}")
                  for m in range(N // 128)]
            qt = [[kpool.tile([128, CH], bf16, tag=f"qt{a}_{j}",
                              name=f"qt{a}_{j}")
                   for j in range(NCHUNK)] for a in range(4)]

            # ---- filler machinery ---------------------------------------
            fillers = deque()  # (emit_fn, est_pe_ns)

            def drain(budget_ns):
                spent = 0
                while fillers and spent < budget_ns:
                    fn, est = fillers.popleft()
                    fn()
                    spent += est

            def drain_all():
                while fillers:
                    fn, _ = fillers.popleft()
                    fn()

            # ---- projection thunks --------------------------------------
            def proj_chain(dst_evac, lhsT_col, j, alloc_ps):
                """Returns thunk fns for one 16-deep contraction chain.

                lhsT_col(t) -> AP for the stationary tile;
                dst_evac(ps) emits the evacuation."""
                ps_box = {}

                def pair(q):
                    def emit():
                        if q == 0:
                            ps_box["ps"] = alloc_ps()
                        ps = ps_box["ps"]
                        for t in range(2 * q, 2 * q + 2):
                            nc.tensor.matmul(
                                ps[:], lhsT_col(t), xt_rhs(t, j),
                                start=(t == 0), stop=(t == NT - 1))
                        if q == 7:
                            dst_evac(ps)
                    return emit
                return [(pair(q), 440) for q in range(8)]

            def proj_thunks(j):
                # j=0 runs before any attention: rotate the 6 chains over
                # the (then free) attention banks and emit chunk-major —
                # with the k/v chains leading the q chains by two steps
                # (wq lands on the scalar ring after wk).
                if j == 0:
                    rots = [
                        lambda: ps_proj.tile([128, CH], f32, tag="proj",
                                             name="pps"),
                        lambda: ps_pa.tile([128, CH], f32, tag="paA",
                                           name="pps"),
                        lambda: ps_pa.tile([128, CH], f32, tag="paB",
                                          name="pps"),
                        lambda: ps_d.tile([128, CH], f32, tag="dn",
                                          name="pps"),
                        lambda: s0_j0[:, 0:CH],
                        lambda: s1_j0[:, 0:CH],
                    ]
                else:
                    rots = [lambda: ps_proj.tile([128, CH], f32, tag="proj",
                                                 name="pps")]
                ch = [0]

                def nxt():
                    a = rots[ch[0] % len(rots)]
                    ch[0] += 1
                    return a
                chains = []

                def kev(ps):
                    nc.vector.tensor_copy(kt[j][:], ps[:])
                chains.append(proj_chain(
                    kev, lambda t: wk_t[:, t * 128:(t + 1) * 128], j, nxt()))

                def vev(ps):
                    vt_s = wpool.tile([128, CH], bf16, tag="vt", name="vt_s")
                    nc.vector.tensor_copy(vt_s[:], ps[:])
                    for s in range(4):
                        nc.sync.dma_start_transpose(
                            v3[4 * j + s][:], vt_s[:, s * 128:(s + 1) * 128])
                chains.append(proj_chain(
                    vev, lambda t: wv_t[:, t * 128:(t + 1) * 128], j, nxt()))
                for a in range(4):
                    def qev(ps, a=a):
                        nc.vector.tensor_copy(qt[a][j][:], ps[:])
                    chains.append(proj_chain(
                        qev, lambda t, a=a: wq_col(t, a), j, nxt()))
                if j == 0:
                    # staggered chunk-major: k/v advance at DMA pace, the
                    # q chains trail by 2 chunk-steps.
                    seq = []
                    for q in range(8):
                        seq.append(chains[0][q])
                        seq.append(chains[1][q])
                        if q >= 2:
                            seq.extend(chains[c][q - 2] for c in range(2, 6))
                    for q in (6, 7):
                        seq.extend(chains[c][q] for c in range(2, 6))
                    return seq
                kv = [th for chain in chains[0:2] for th in chain]
                qs = [th for chain in chains[2:6] for th in chain]
                return kv, qs

            # ---- o_proj thunks ------------------------------------------
            # last chunk: the a=0,1 half-contraction runs as fillers
            # during g=1 of attn(3) and streams straight to DRAM; the
            # tail computes a=2,3 and ACCUMULATES into the same DRAM rows
            # via gpsimd DMA (same ring -> ordered), halving the
            # post-attention PE run.
            def oproj_half_thunks(ci, an_tiles):
                n0 = ci * CH
                th = []
                st_box = {}
                for nt_ in range(4):
                    for dc in range(4):
                        def emit(nt_=nt_, dc=dc):
                            po = ps_proj.tile([128, CH], f32, tag="proj",
                                              name="poh")
                            for a in range(2):
                                nc.tensor.matmul(
                                    po[:],
                                    an_tiles[a][:, nt_ * 128:(nt_ + 1) * 128],
                                    wo_t[:, a * D + dc * CH:
                                         a * D + (dc + 1) * CH],
                                    start=(a == 0), stop=(a == 1))
                            if dc == 0:
                                st_box[nt_] = wpool.tile(
                                    [128, 2048], bf16, tag="st", bufs=2,
                                    name="sth")
                            st = st_box[nt_]
                            nc.vector.tensor_copy(
                                st[:, dc * CH:(dc + 1) * CH], po[:])
                            if dc == 3:
                                r0 = n0 + nt_ * 128
                                nc.gpsimd.dma_start(
                                    part.ap()[r0:r0 + 128, :], st[:])
                        th.append((emit, 470))
                return th

            def oproj_thunks(ci, an_tiles, tail=False):
                n0 = ci * CH
                th = []
                # in the tail (post-attention) phase all attention banks
                # are free: rotate po over 6 slots so the PE never waits
                # on a single bank's evacuation.
                tail_rots = [
                    lambda: ps_proj.tile([128, CH], f32, tag="proj",
                                         name="po"),
                    lambda: ps_pa.tile([128, CH], f32, tag="paA", name="po"),
                    lambda: ps_pa.tile([128, CH], f32, tag="paB", name="po"),
                    lambda: ps_d.tile([128, CH], f32, tag="dn", name="po"),
                    lambda: ps_s.tile([128, 1024], f32, tag="s0",
                                      name="po")[:, 0:CH],
                    lambda: ps_s.tile([128, 1024], f32, tag="s1",
                                      name="po")[:, 0:CH],
                ]
                st_box = {}
                for nt_ in range(4):
                    for dc in range(4):
                        def emit(nt_=nt_, dc=dc):
                            if tail:
                                po = tail_rots[(nt_ * 4 + dc) % 6]()
                            else:
                                po = ps_proj.tile([128, CH], f32, tag="proj",
                                                  name="po")
                            for a in range(4):
                                nc.tensor.matmul(
                                    po[:],
                                    an_tiles[a][:, nt_ * 128:(nt_ + 1) * 128],
                                    wo_t[:, a * D + dc * CH:
                                         a * D + (dc + 1) * CH],
                                    start=(a == 0), stop=(a == 3))
                            if dc == 0:
                                st_box[nt_] = wpool.tile(
                                    [128, 2048], bf16, tag="st", bufs=2,
                                    name="st")
                            st = st_box[nt_]
                            dst = st[:, dc * CH:(dc + 1) * CH]
                            # tail: split the copy across scalar+vector so
                            # the final accumulate-DMAs fire early;
                            # mid-kernel: keep scalar free for the EXPs.
                            if tail:
                                nc.scalar.copy(dst[0:64, :], po[0:64, :])
                                nc.vector.tensor_copy(dst[64:128, :],
                                                      po[64:128, :])
                            else:
                                nc.vector.tensor_copy(dst, po[:])
                            if dc == 3:
                                r0 = n0 + nt_ * 128
                                nc.gpsimd.dma_start(
                                    part.ap()[r0:r0 + 64, :],
                                    st[0:64, :])
                                nc.sync.dma_start(
                                    part.ap()[r0 + 64:r0 + 128, :],
                                    st[64:128, :])
                        th.append((emit, 880))
                return th

            # ---- attention ----------------------------------------------
            def attn_chunk(ci, mid_hook=None):
                M = 4 * ci + 4
                an_tiles = []
                for g in range(2):
                    # spread the queued filler work evenly over the
                    # remaining drain points (one per (g, mt) block) so
                    # fillers run in long PE-dense runs and never dry up
                    # mid-chunk.
                    tot_est = sum(e for _, e in fillers)
                    per_block = tot_est / ((2 - g) * M) if tot_est else 0
                    a0, a1 = 2 * g, 2 * g + 1
                    paA = ps_pa.tile([128, CH], f32, tag="paA", name="paA")
                    paB = ps_pa.tile([128, CH], f32, tag="paB", name="paB")
                    dn = ps_d.tile([128, CH], f32, tag="dn", name="dn")
                    pts = {}

                    def scores_exp(mt):
                        # score layout per head pair a: kv0 block at cols
                        # [0:F], kv1 block at cols [512:512+F] (bank 2);
                        # one FD=512+F exp covers both (cols [F:512] are
                        # junk for diagonal tiles and never read).
                        jmt, cmt = mt // 4, mt % 4
                        flo = max(0, (mt - 4 * ci) * 128)
                        F = CH - flo
                        # kv1 block always in bank 1: the two row-tiled
                        # score matmuls run concurrently and must not
                        # target the same PSUM bank (cols [F:CH] junk)
                        off = CH
                        sss, pt_pair = [], []
                        for i, a in enumerate((a0, a1)):
                            ss = ps_s.tile([128, 1024], f32, tag=f"s{i}",
                                           name="ss")
                            nc.tensor.matmul(
                                ss[:, 0:F],
                                kt[jmt][0:64, cmt * 128:(cmt + 1) * 128],
                                qt[a][ci][0:64, flo:CH],
                                start=True, stop=True, tile_position=(0, 0))
                            nc.tensor.matmul(
                                ss[:, off:off + F],
                                kt[jmt][64:128, cmt * 128:(cmt + 1) * 128],
                                qt[a][ci][64:128, flo:CH],
                                start=True, stop=True, tile_position=(64, 0))
                            sss.append(ss)
                        for i in range(2):
                            pt_ = wpool.tile([128, 1024], bf16, tag=f"pt{i}",
                                             name="pt")
                            if F == CH:
                                nc.scalar.activation(pt_[:, 0:off + F],
                                                     sss[i][:, 0:off + F],
                                                     Exp, scale=0.125)
                            else:
                                # diagonal tile: strided AP skips the junk
                                # gap [F:512] so the exp covers only the
                                # two live blocks (scalar chain is the
                                # block-rate limiter)
                                sa, pa_ = sss[i][:], pt_[:]
                                src = AP(sa.tensor, sa.offset,
                                         [[sa.ap[0][0], 128],
                                          [CH, 2], [1, F]])
                                dst = AP(pa_.tensor, pa_.offset,
                                         [[pa_.ap[0][0], 128],
                                          [CH, 2], [1, F]])
                                nc.scalar.activation(dst, src,
                                                     Exp, scale=0.125)
                            if mt >= 4 * ci:  # diagonal block: causal mask
                                pta = pt_[:]
                                pv = AP(pta.tensor, pta.offset,
                                        [[1024, 128], [512, 2], [1, 128]])
                                ma = msk_t[:]
                                mv = AP(ma.tensor, ma.offset,
                                        [[512, 128], [128, 2], [1, 128]])
                                nc.vector.tensor_mul(pv, pv, mv)
                            pt_pair.append(pt_)
                        pts[mt] = (pt_pair, flo, F, off)

                    def av_denom(mt):
                        pt_pair, flo, F, off = pts.pop(mt)
                        first, last = (mt == 0), (mt == M - 1)
                        for i in range(2):
                            pa = paA if i == 0 else paB
                            pt_ = pt_pair[i]
                            nc.tensor.matmul(
                                pa[0:64, flo:CH], v3[mt][:, 0:64],
                                pt_[:, 0:F],
                                start=first, stop=last, tile_position=(0, 0))
                            nc.tensor.matmul(
                                pa[64:128, flo:CH], v3[mt][:, 64:128],
                                pt_[:, off:off + F],
                                start=first, stop=last, tile_position=(0, 64))
                        for pos, src in ((0, pt_pair[0][:, 0:F]),
                                         (32, pt_pair[0][:, off:off + F]),
                                         (64, pt_pair[1][:, 0:F]),
                                         (96, pt_pair[1][:, off:off + F])):
                            nc.tensor.matmul(
                                dn[pos:pos + 1, flo:CH], ones1[:], src,
                                start=first, stop=last,
                                tile_position=(0, pos))

                    for mt in range(M):
                        scores_exp(mt)
                        if mt > 0:
                            av_denom(mt - 1)
                        drain(max(900, per_block))
                    av_denom(M - 1)

                    # Evacuate pa raw (frees the accumulator banks for the
                    # next group immediately); normalize off-critical-path.
                    aots = []
                    for i in range(2):
                        aot = wpool.tile([128, CH], bf16, tag=f"aot{i}",
                                         name="aot")
                        nc.vector.tensor_copy(aot[:], (paA if i == 0
                                                       else paB)[:])
                        aots.append(aot)
                    d4r = wpool.tile([128, CH], f32, tag="d4r", name="d4r")
                    nc.vector.reciprocal_approx_fast(d4r[0:97, :], dn[0:97, :])
                    # broadcast 1/denom rows to 128 partitions via K=1 PE
                    # matmuls into the (now free) score banks: all four in
                    # one window (pairwise-disjoint PE quadrants)
                    rbs = [ps_s.tile([128, 1024], f32, tag=f"s{i}",
                                     name="rb") for i in range(2)]
                    for i in range(2):
                        r0, r1 = 64 * i, 64 * i + 32
                        nc.tensor.matmul(rbs[i][0:64, 0:CH],
                                         onesb[r0:r0 + 1, :],
                                         d4r[r0:r0 + 1, :], start=True,
                                         stop=True, tile_position=(r0, 0))
                        nc.tensor.matmul(rbs[i][64:128, CH:2 * CH],
                                         onesb[r1:r1 + 1, :],
                                         d4r[r1:r1 + 1, :], start=True,
                                         stop=True, tile_position=(r1, 64))
                    for i, a in enumerate((a0, a1)):
                        rbr = wpool.tile([128, CH], f32, tag=f"rbr{i}",
                                         name="rbr")
                        nc.vector.tensor_copy(rbr[0:64, :],
                                              rbs[i][0:64, 0:CH])
                        nc.vector.tensor_copy(rbr[64:128, :],
                                              rbs[i][64:128, CH:2 * CH])
                        an = wpool.tile([128, CH], bf16, tag=f"an{a}",
                                        name=f"an{a}")
                        nc.vector.tensor_mul(an[:], aots[i][:], rbr[:])
                        an_tiles.append(an)
                    if g == 0 and mid_hook is not None:
                        fillers.extend(mid_hook(an_tiles))
                return an_tiles

            # ---- main schedule ------------------------------------------
            for fn, _ in proj_thunks(0):
                fn()
            def mix3(pth, oth):
                # interleave proj and oproj fillers ~3:1 so each oproj
                # emit's PSUM evacuation hides under proj matmuls instead
                # of stalling the single o_proj bank.
                mix, pi, oi = [], 0, 0
                while pi < len(pth) or oi < len(oth):
                    for _ in range(3):
                        if pi < len(pth):
                            mix.append(pth[pi])
                            pi += 1
                    if oi < len(oth):
                        mix.append(oth[oi])
                        oi += 1
                return mix

            # filler placement: q-chains of chunk j must finish before
            # attn(j) starts, but the k/v outputs of chunk 3 are only
            # consumed from mt=12 onward — defer them into ci=3, which is
            # otherwise filler-poor (exp-gated blocks would starve the PE).
            kv1, qs1 = proj_thunks(1)
            kv2, qs2 = proj_thunks(2)
            kv3, qs3 = proj_thunks(3)
            an_by_ci = {}
            for ci in range(NCHUNK):
                if ci == 0:
                    fillers.extend(mix3(qs1 + kv1, []))
                elif ci == 1:
                    fillers.extend(mix3(qs2 + kv2,
                                        oproj_thunks(0, an_by_ci[0])))
                elif ci == 2:
                    fillers.extend(mix3(qs3,
                                        oproj_thunks(1, an_by_ci[1])))
                else:
                    fillers.extend(mix3(kv3,
                                        oproj_thunks(2, an_by_ci[2])))
                an_by_ci[ci] = attn_chunk(ci)
                drain_all()
            for fn, _ in oproj_thunks(NCHUNK - 1, an_by_ci[NCHUNK - 1],
                                      tail=True):
                fn()
    nc.compile()
    return nc


def _prep_in_maps(x, Wq, Wk, Wv, Wo):
    import jax.numpy as jnp

    def to_bf16(a):
        return np.asarray(jnp.asarray(np.asarray(a), dtype=jnp.bfloat16))

    i = np.arange(128)[:, None]
    j = np.arange(128)[None, :]
    msk = np.tile((i <= j).astype(np.float32), (1, 4))

    def devlay(a):
        # [K*128, O] -> [128, K*O] partition-major device layout
        k = a.shape[0] // 128
        return a.reshape(k, 128, a.shape[1]).transpose(1, 0, 2).reshape(128, -1)

    in_maps = []
    for c in range(N_CORES):
        b, g = c // 4, c % 4
        qh = [8 * g + a for a in range(8)]
        wq_cols = []
        for a in range(4):
            wq_cols.append(np.arange(qh[a] * HD, (qh[a] + 1) * HD))
            wq_cols.append(np.arange(qh[a + 4] * HD, (qh[a + 4] + 1) * HD))
        wq_r = np.asarray(Wq)[:, np.concatenate(wq_cols)]
        wo_r = np.asarray(Wo)[np.concatenate(wq_cols), :]
        wk_s = np.asarray(Wk)[:, 2 * g * HD: (2 * g + 2) * HD]
        wv_s = np.asarray(Wv)[:, 2 * g * HD: (2 * g + 2) * HD]
        in_maps.append({
            "xT": to_bf16(devlay(np.ascontiguousarray(np.asarray(x)[b].T))),
            "wqA": to_bf16(devlay(wq_r[:, 0:256])),
            "wqB": to_bf16(devlay(wq_r[:, 256:512])),
            "wk": to_bf16(devlay(wk_s)),
            "wv": to_bf16(devlay(wv_s)),
            "wo": to_bf16(devlay(wo_r)),
            "msk": to_bf16(msk),
        })
    return in_maps


def kernel(x, Wq, Wk, Wv, Wo, trace=False):
    if "nc" not in _CACHE:
        _CACHE["nc"] = _build()
    nc = _CACHE["nc"]
    in_maps = _prep_in_maps(x, Wq, Wk, Wv, Wo)
    res = bass_utils.run_bass_kernel_spmd(
        nc, in_maps, core_ids=list(range(N_CORES)), trace=trace)
    _CACHE["last_result"] = res
    out = np.zeros((B, N, D), np.float32)
    for c in range(N_CORES):
        out[c // 4] += np.asarray(res.results[c]["part"], dtype=np.float32)
    return out
